# revision 29
# baseline (speedup 1.0000x reference)
"""Causal multi-head attention on 8 Trainium2 NeuronCores.

Problem: B=2, NH=16, T=2048, D=64 fp32.
Sharding: the 32 (batch, head) pairs are split 4-per-core; each core runs its
heads' full causal attention independently (no collectives).

Per-core kernel design (per head):
  - S^T blocks [k=128 partitions, q free] = K_blk @ Q^T via PE (float32r).
    Causality at 128-row granularity: iteration kb only computes q >= 128*kb.
  - Diagonal 128x128 block gets an additive -1e9 upper-strict-triangle mask
    (DVE, in-place in PSUM).
  - exp(S/8) on ScalarE, PSUM -> SBUF (this doubles as the PSUM evacuation).
  - O^T accumulation: PSUM [65, q] += [V | ones]^T_blk @ P^T_blk. Row 64 is
    the softmax denominator (free).
  - Epilogue: copy O^T to SBUF, PE-transpose 128-col chunks back to [q, 65],
    multiply by reciprocal of the sums column (DVE), DMA out.

The host side only reformats layouts (transpose/pack/shard in numpy); every
FLOP of the attention math runs on device.
"""

import numpy as np

import concourse.mybir as mybir
import concourse.tile as tile
from concourse import bacc
from concourse.bass_utils import run_bass_kernel_spmd

B, NH, T, D = 2, 16, 2048, 64
HPC = 4  # heads per core
NCORES = 8
NKB = T // 128  # 16 k-blocks of 128 rows
F32 = mybir.dt.float32
F32R = mybir.dt.float32r
NEG = -1.0e9

_cached = {}


def _build(reps=1):
    key = ("nc", reps)
    if key in _cached:
        return _cached[key]
    nc = bacc.Bacc("TRN2", target_bir_lowering=False, debug=False)
    # Q^T / K^T: [64, T] (d on partitions)
    qt = nc.dram_tensor("qt", (HPC, D, T), F32R, kind="ExternalInput").ap()
    kt = nc.dram_tensor("kt", (HPC, D, T), F32R, kind="ExternalInput").ap()
    # V augmented with a ones column: [h, p, c, d] = V[h, 128*c + p, d], d=64 -> 1.0
    v = nc.dram_tensor("v", (HPC, 128, NKB, D + 1), F32R, kind="ExternalInput").ap()
    mask = nc.dram_tensor("mask", (128, 128), F32, kind="ExternalInput").ap()
    ident = nc.dram_tensor("ident", (128, 128), F32, kind="ExternalInput").ap()
    # out [h, p, c*64 + d] = O[h, 128*c + p, d]
    o = nc.dram_tensor("o", (HPC, 128, NKB * D), F32, kind="ExternalOutput").ap()

    EXP = mybir.ActivationFunctionType.Exp

    with tile.TileContext(nc) as tc:
        with (
            tc.tile_pool(name="constp", bufs=1) as constp,
            tc.tile_pool(name="qkp", bufs=2) as qkp,
            tc.tile_pool(name="ptp", bufs=4) as ptp,
            tc.tile_pool(name="osbp", bufs=2) as osbp,
            tc.tile_pool(name="spp", bufs=3, space="PSUM") as spp,
            tc.tile_pool(name="opp", bufs=1, space="PSUM") as opp,
        ):
            mask_sb = constp.tile([128, 128], F32)
            nc.sync.dma_start(mask_sb[:], mask[:])
            id_sb = constp.tile([128, 128], F32)
            nc.sync.dma_start(id_sb[:], ident[:])

            def body():
                _emit_body(nc, tc, qt, kt, v, o, mask_sb, id_sb, qkp, ptp, osbp, spp, opp)

            if reps == 1:
                body()
            else:
                with tc.For_i(0, reps, 1):
                    body()

    nc.compile()
    _cached[key] = nc
    return nc


def _emit_body(nc, tc, qt, kt, v, o, mask_sb, id_sb, qkp, ptp, osbp, spp, opp):
    """Software-pipelined emission.

    Each head's q range is processed in two 1024-column passes (A: q<1024,
    B: q>=1024) so the O^T accumulator needs only 2 PSUM banks, leaving 6
    banks for three 1024-wide S^T tiles (pipeline depth 3). S matmuls run
    three chunks ahead of exp/O in the static schedule so ScalarE (the
    bottleneck engine) never waits on PE. The normalize/transpose epilogue is
    emitted per finished 512-column bank so it overlaps remaining k-blocks.
    """
    EXP = mybir.ActivationFunctionType.Exp

    chunks = []  # (h, g, kb): g = 1024-col block (0: pass A, 1: pass B)
    for h in range(HPC):
        for kb in range(8):
            chunks.append((h, 0, kb))
        for kb in range(NKB):
            chunks.append((h, 1, kb))
    n = len(chunks)

    sb = {}  # h -> (qt_sb, kt_sb, v_sb)
    heads = {}  # h -> dict
    oaccs = {}  # (h, g) -> psum tile [65, 1024]
    sch_tiles = {}  # chunk idx -> (sch, lo, c0, c1)

    # Warm the ACT exp table immediately (overlaps the first input DMAs).
    warm = osbp.tile([128, 1], F32, tag="warm")
    nc.scalar.activation(warm[:], mask_sb[:, :1], EXP, scale=0.0)

    def load(h, first=False):
        # First two heads load on the sync (HWDGE) queue — fast startup, the
        # store queue is empty anyway. Later prefetches go on the gpsimd
        # (SWDGE) queue so they never head-of-line-block the output stores.
        eng = nc.sync if first else nc.gpsimd
        qt_sb = qkp.tile([D, T], F32R, tag="qt", name=f"qt_sb{h}")
        kt_sb = qkp.tile([D, T], F32R, tag="kt", name=f"kt_sb{h}")
        v_sb = qkp.tile([128, NKB, D + 1], F32R, tag="v", name=f"v_sb{h}")
        if first:
            # fine-grained leading pieces so S(0) unblocks ASAP
            eng.dma_start(kt_sb[:, :128], kt[h, :, :128])
            eng.dma_start(qt_sb[:, :512], qt[h, :, :512])
            eng.dma_start(qt_sb[:, 512 : T // 2], qt[h, :, 512 : T // 2])
            eng.dma_start(kt_sb[:, 128 : T // 2], kt[h, :, 128 : T // 2])
        else:
            eng.dma_start(kt_sb[:, : T // 2], kt[h, :, : T // 2])
            eng.dma_start(qt_sb[:, : T // 2], qt[h, :, : T // 2])
        eng.dma_start(v_sb[:, : NKB // 2], v[h, :, : NKB // 2])
        eng.dma_start(kt_sb[:, T // 2 :], kt[h, :, T // 2 :])
        eng.dma_start(qt_sb[:, T // 2 :], qt[h, :, T // 2 :])
        eng.dma_start(v_sb[:, NKB // 2 :], v[h, :, NKB // 2 :])
        sb[h] = (qt_sb, kt_sb, v_sb)

    def emit_S(i):
        h, g, kb = chunks[i]
        qt_sb, kt_sb, _ = sb[h]
        qs = kb * 128
        c0 = max(qs, 1024 * g)
        c1 = 1024 * (g + 1)
        lo = c0 - 1024 * g
        sch = spp.tile([128, 1024], F32, tag="s", name=f"sch{i}")
        first = True
        p = c0
        while p < c1:
            pe = min(c1, (p // 512 + 1) * 512)
            nc.tensor.matmul(
                sch[:, p - 1024 * g : pe - 1024 * g],
                lhsT=kt_sb[:, qs : qs + 128],
                rhs=qt_sb[:, p:pe],
                start=True,
                stop=True,
            )
            if first and c0 == qs:
                nc.vector.tensor_add(
                    sch[:, lo : lo + 128], sch[:, lo : lo + 128], mask_sb[:]
                )
            first = False
            p = pe
        sch_tiles[i] = (sch, lo, c0, c1)

    def emit_exp_O(i):
        h, g, kb = chunks[i]
        _, _, v_sb = sb[h]
        sch, lo, c0, c1 = sch_tiles.pop(i)
        ptt = ptp.tile([128, 1024], F32R, tag="pt", name=f"ptt{i}")
        nc.scalar.activation(ptt[:, lo:1024], sch[:, lo:1024], EXP, scale=0.125)
        if (h, g) not in oaccs:
            # 128 partitions: rows 0-64 hold O^T+sums; the in-place transpose
            # in emit_bank_norm reuses the retired bank with all 128 rows.
            oaccs[(h, g)] = opp.tile(
                [128, 1024], F32, tag="oacc", name=f"oacc{h}_{g}"
            )
        if h not in heads:
            heads[h] = {
                "ot": osbp.tile([D + 1, T], F32, tag="ot", name=f"ot_sb{h}"),
                "o": osbp.tile([128, NKB * D], F32, tag="o", name=f"o_sb{h}"),
                "rec": osbp.tile([128, NKB], F32, tag="rec", name=f"rec{h}"),
            }
        oacc = oaccs[(h, g)]
        p = c0
        while p < c1:
            pe = min(c1, (p // 512 + 1) * 512)
            nc.tensor.matmul(
                oacc[: D + 1, p - 1024 * g : pe - 1024 * g],
                lhsT=v_sb[:, kb, :],
                rhs=ptt[:, p - 1024 * g : pe - 1024 * g],
                start=(kb == 0),
                stop=(kb == (pe - 1) // 128),
                skip_group_check=True,
            )
            p = pe

    def emit_bank_copy(h, b):
        # global bank b (cols [512b, 512b+512)) is final; evacuate to SBUF.
        hd = heads[h]
        oacc = oaccs[(h, b // 2)]
        lb = 512 * (b % 2)
        nc.vector.tensor_copy(
            hd["ot"][:, 512 * b : 512 * (b + 1)], oacc[: D + 1, lb : lb + 512]
        )

    def emit_bank_norm(h, b):
        # Transpose back to [q, 65], then normalize by the sums column and
        # store. Mid-pass banks (b even) transpose IN-PLACE into the retired
        # oacc bank (no PSUM slot stolen from the S pipeline); pass-end banks
        # (b odd) use an S-pool slot so the oacc slot frees at the copy and
        # the next pass's O accumulation can start immediately.
        hd = heads[h]
        ot_sb, o_sb, rec = hd["ot"], hd["o"], hd["rec"]
        if b % 2 == 0:
            oacc = oaccs[(h, b // 2)]
            lb = 512 * (b % 2)
            pso = oacc[:, lb : lb + 4 * 65]
        else:
            pso = spp.tile([128, 1024], F32, tag="s", name=f"pso{h}_{b}")[
                :, : 4 * 65
            ]
        for j in range(4):
            c = 4 * b + j
            nc.tensor.transpose(
                pso[:, 65 * j : 65 * j + 65],
                ot_sb[:, 128 * c : 128 * c + 128],
                id_sb[:65, :65],
            )
        sums = pso.rearrange("p (c d) -> p c d", d=65)[:, :, 64]
        nc.vector.reciprocal(rec[:, 4 * b : 4 * b + 4], sums)
        for j in range(4):
            c = 4 * b + j
            nc.vector.tensor_scalar_mul(
                o_sb[:, 64 * c : 64 * c + 64],
                pso[:, 65 * j : 65 * j + 64],
                rec[:, c : c + 1],
            )
        nc.sync.dma_start(
            o[h, :, 256 * b : 256 * (b + 1)], o_sb[:, 256 * b : 256 * (b + 1)]
        )

    LOOKAHEAD = 3
    load(0, first=True)
    load(1, first=True)
    for i in range(min(LOOKAHEAD, n)):
        emit_S(i)
    deferred = {}  # emit-at chunk idx -> (h, b) norm work
    for i in range(n):
        h, g, kb = chunks[i]
        emit_exp_O(i)
        if i + LOOKAHEAD < n:
            emit_S(i + LOOKAHEAD)
        if i in deferred:
            emit_bank_norm(*deferred.pop(i))
        # Bank completion: pass A finishes banks 0 (kb=3) and 1 (kb=7);
        # pass B finishes banks 2 (kb=11) and 3 (kb=15).
        if kb % 4 == 3 and (g == 0 or kb >= 8):
            b = kb // 4
            emit_bank_copy(h, b)
            deferred[min(i + 2, n - 1)] = (h, b)
            if b == 3 and h + 2 < HPC:
                load(h + 2)
    for i in sorted(deferred):
        emit_bank_norm(*deferred[i])
    deferred.clear()


def _prep_in_maps(Q, K, V):
    Q = np.asarray(Q, dtype=np.float32).reshape(B * NH, T, D)
    K = np.asarray(K, dtype=np.float32).reshape(B * NH, T, D)
    V = np.asarray(V, dtype=np.float32).reshape(B * NH, T, D)

    mask = np.where(
        np.arange(128)[:, None] <= np.arange(128)[None, :], 0.0, NEG
    ).astype(np.float32)
    ident = np.eye(128, dtype=np.float32)

    in_maps = []
    for c in range(NCORES):
        hs = slice(HPC * c, HPC * (c + 1))
        qt = Q[hs].transpose(0, 2, 1)  # [hpc, 64, T]
        kt = K[hs].transpose(0, 2, 1)
        va = np.concatenate(
            [V[hs], np.ones((HPC, T, 1), dtype=np.float32)], axis=-1
        )  # [hpc, T, 65]
        va = va.reshape(HPC, NKB, 128, D + 1).transpose(0, 2, 1, 3)  # [hpc,128,16,65]
        in_maps.append(
            {
                "qt": np.ascontiguousarray(qt),
                "kt": np.ascontiguousarray(kt),
                "v": np.ascontiguousarray(va),
                "mask": mask,
                "ident": ident,
            }
        )
    return in_maps


def _gather(results):
    out = np.empty((B * NH, T, D), dtype=np.float32)
    for c in range(NCORES):
        oc = results[c]["o"]  # [HPC, 128, NKB*D]
        for s in range(HPC):
            out[HPC * c + s] = (
                oc[s].reshape(128, NKB, D).transpose(1, 0, 2).reshape(T, D)
            )
    return out.reshape(B, NH, T, D)


def _run(in_maps, **kwargs):
    nc = _build()
    return run_bass_kernel_spmd(nc, in_maps, core_ids=list(range(NCORES)), **kwargs)


def kernel(Q, K, V):
    in_maps = _prep_in_maps(Q, K, V)
    res = _run(in_maps)
    return _gather(res.results)


# revision 35
# speedup vs baseline: 85.4520x; 85.4520x over previous
"""Causal multi-head attention on 8 Trainium2 NeuronCores.

Problem: B=2, NH=16, T=2048, D=64 fp32.
Sharding: the 32 (batch, head) pairs are split 4-per-core; each core runs its
heads' full causal attention independently (no collectives).

Per-core kernel design (per head):
  - S^T blocks [k=128 partitions, q free] = K_blk @ Q^T via PE (float32r).
    Causality at 128-row granularity: iteration kb only computes q >= 128*kb.
  - Diagonal 128x128 block gets an additive -1e9 upper-strict-triangle mask
    (DVE, in-place in PSUM).
  - exp(S/8) on ScalarE, PSUM -> SBUF (this doubles as the PSUM evacuation).
  - O^T accumulation: PSUM [65, q] += [V | ones]^T_blk @ P^T_blk. Row 64 is
    the softmax denominator (free).
  - Epilogue: copy O^T to SBUF, PE-transpose 128-col chunks back to [q, 65],
    multiply by reciprocal of the sums column (DVE), DMA out.

The host side only reformats layouts (transpose/pack/shard in numpy); every
FLOP of the attention math runs on device.
"""

import numpy as np

import concourse.mybir as mybir
import concourse.tile as tile
from concourse import bacc
from concourse.bass_utils import run_bass_kernel_spmd

B, NH, T, D = 2, 16, 2048, 64
HPC = 4  # heads per core
NCORES = 8
NKB = T // 128  # 16 k-blocks of 128 rows
F32 = mybir.dt.float32
F32R = mybir.dt.float32r
NEG = -1.0e9

_cached = {}


def _build(reps=1):
    key = ("nc", reps)
    if key in _cached:
        return _cached[key]
    nc = bacc.Bacc("TRN2", target_bir_lowering=False, debug=False)
    # Q^T / K^T: [64, T] (d on partitions)
    qt = nc.dram_tensor("qt", (HPC, D, T), F32R, kind="ExternalInput").ap()
    kt = nc.dram_tensor("kt", (HPC, D, T), F32R, kind="ExternalInput").ap()
    # V augmented with a ones column: [h, p, c, d] = V[h, 128*c + p, d], d=64 -> 1.0
    v = nc.dram_tensor("v", (HPC, 128, NKB, D + 1), F32R, kind="ExternalInput").ap()
    mask = nc.dram_tensor("mask", (128, 128), F32, kind="ExternalInput").ap()
    ident = nc.dram_tensor("ident", (128, 128), F32, kind="ExternalInput").ap()
    # out [h, p, c*64 + d] = O[h, 128*c + p, d]
    o = nc.dram_tensor("o", (HPC, 128, NKB * D), F32, kind="ExternalOutput").ap()

    EXP = mybir.ActivationFunctionType.Exp

    with tile.TileContext(nc) as tc:
        with (
            tc.tile_pool(name="constp", bufs=1) as constp,
            tc.tile_pool(name="qkp", bufs=2) as qkp,
            tc.tile_pool(name="ptp", bufs=4) as ptp,
            tc.tile_pool(name="osbp", bufs=2) as osbp,
            tc.tile_pool(name="spp", bufs=3, space="PSUM") as spp,
            tc.tile_pool(name="opp", bufs=2, space="PSUM") as opp,
        ):
            mask_sb = constp.tile([128, 128], F32)
            nc.sync.dma_start(mask_sb[:], mask[:])
            id_sb = constp.tile([128, 128], F32)
            nc.sync.dma_start(id_sb[:], ident[:])

            def body():
                _emit_body(nc, tc, qt, kt, v, o, mask_sb, id_sb, qkp, ptp, osbp, spp, opp)

            if reps == 1:
                body()
            else:
                with tc.For_i(0, reps, 1):
                    body()

    nc.compile()
    _cached[key] = nc
    return nc


def _emit_body(nc, tc, qt, kt, v, o, mask_sb, id_sb, qkp, ptp, osbp, spp, opp):
    """Software-pipelined emission.

    Each head's q range is processed in two 1024-column passes (A: q<1024,
    B: q>=1024) so the O^T accumulator needs only 2 PSUM banks, leaving 6
    banks for three 1024-wide S^T tiles (pipeline depth 3). S matmuls run
    three chunks ahead of exp/O in the static schedule so ScalarE (the
    bottleneck engine) never waits on PE. The normalize/transpose epilogue is
    emitted per finished 512-column bank so it overlaps remaining k-blocks.
    """
    EXP = mybir.ActivationFunctionType.Exp

    chunks = []  # (h, g, kb): g = 1024-col block (0: pass A, 1: pass B)
    for h in range(HPC):
        for kb in range(8):
            chunks.append((h, 0, kb))
        for kb in range(NKB):
            chunks.append((h, 1, kb))
    n = len(chunks)

    sb = {}  # h -> (qt_sb, kt_sb, v_sb)
    heads = {}  # h -> dict
    oaccs = {}  # (h, g) -> psum tile [65, 1024]
    sch_tiles = {}  # chunk idx -> (sch, lo, c0, c1)

    # Warm the ACT exp table immediately (overlaps the first input DMAs).
    warm = osbp.tile([128, 1], F32, tag="warm")
    nc.scalar.activation(warm[:], mask_sb[:, :1], EXP, scale=0.0)

    def load(h, first=False):
        # First two heads load on the sync (HWDGE) queue — fast startup, the
        # store queue is empty anyway. Later prefetches go on the gpsimd
        # (SWDGE) queue so they never head-of-line-block the output stores.
        eng = nc.sync if first else nc.gpsimd
        qt_sb = qkp.tile([D, T], F32R, tag="qt", name=f"qt_sb{h}")
        kt_sb = qkp.tile([D, T], F32R, tag="kt", name=f"kt_sb{h}")
        v_sb = qkp.tile([128, NKB, D + 1], F32R, tag="v", name=f"v_sb{h}")
        if first:
            # fine-grained leading pieces on the empty sync queue so S(0)
            # unblocks ASAP; the rest on the gpsimd queue in parallel.
            nc.sync.dma_start(kt_sb[:, :128], kt[h, :, :128])
            nc.sync.dma_start(qt_sb[:, :512], qt[h, :, :512])
            nc.gpsimd.dma_start(qt_sb[:, 512 : T // 2], qt[h, :, 512 : T // 2])
            nc.gpsimd.dma_start(kt_sb[:, 128 : T // 2], kt[h, :, 128 : T // 2])
            eng = nc.gpsimd
        else:
            eng.dma_start(kt_sb[:, : T // 2], kt[h, :, : T // 2])
            eng.dma_start(qt_sb[:, : T // 2], qt[h, :, : T // 2])
        eng.dma_start(v_sb[:, : NKB // 2], v[h, :, : NKB // 2])
        eng.dma_start(kt_sb[:, T // 2 :], kt[h, :, T // 2 :])
        eng.dma_start(qt_sb[:, T // 2 :], qt[h, :, T // 2 :])
        eng.dma_start(v_sb[:, NKB // 2 :], v[h, :, NKB // 2 :])
        sb[h] = (qt_sb, kt_sb, v_sb)

    def emit_S(i):
        h, g, kb = chunks[i]
        qt_sb, kt_sb, _ = sb[h]
        qs = kb * 128
        c0 = max(qs, 1024 * g)
        c1 = 1024 * (g + 1)
        lo = c0 - 1024 * g
        sch = spp.tile([128, 1024], F32, tag="s", name=f"sch{i}")
        first = True
        p = c0
        while p < c1:
            pe = min(c1, (p // 512 + 1) * 512)
            nc.tensor.matmul(
                sch[:, p - 1024 * g : pe - 1024 * g],
                lhsT=kt_sb[:, qs : qs + 128],
                rhs=qt_sb[:, p:pe],
                start=True,
                stop=True,
            )
            if first and c0 == qs:
                nc.vector.tensor_add(
                    sch[:, lo : lo + 128], sch[:, lo : lo + 128], mask_sb[:]
                )
            first = False
            p = pe
        sch_tiles[i] = (sch, lo, c0, c1)

    def emit_exp_O(i):
        h, g, kb = chunks[i]
        _, _, v_sb = sb[h]
        sch, lo, c0, c1 = sch_tiles.pop(i)
        ptt = ptp.tile([128, 1024], F32R, tag="pt", name=f"ptt{i}")
        nc.scalar.activation(ptt[:, lo:1024], sch[:, lo:1024], EXP, scale=0.125)
        if h not in heads:
            heads[h] = {
                "ot": osbp.tile([D + 1, T], F32, tag="ot", name=f"ot_sb{h}"),
                "o": osbp.tile([128, NKB * D], F32, tag="o", name=f"o_sb{h}"),
                "rec": osbp.tile([128, NKB], F32, tag="rec", name=f"rec{h}"),
            }
        p = c0
        while p < c1:
            pe = min(c1, (p // 512 + 1) * 512)
            b = p // 512  # global 512-col bank; one PSUM tile per bank so a
            # retired bank's in-place norm never aliases the live bank
            if (h, b) not in oaccs:
                # 128 partitions: rows 0-64 hold O^T+sums; emit_bank_norm's
                # in-place transpose reuses the retired bank with all 128.
                oaccs[(h, b)] = opp.tile(
                    [128, 512], F32, tag="oacc", name=f"oacc{h}_{b}"
                )
            nc.tensor.matmul(
                oaccs[(h, b)][: D + 1, p - 512 * b : pe - 512 * b],
                lhsT=v_sb[:, kb, :],
                rhs=ptt[:, p - 1024 * g : pe - 1024 * g],
                start=(kb == 0),
                stop=(kb == (pe - 1) // 128),
                skip_group_check=True,
            )
            p = pe

    def emit_bank_copy(h, b):
        # global bank b (cols [512b, 512b+512)) is final; evacuate to SBUF.
        hd = heads[h]
        nc.vector.tensor_copy(
            hd["ot"][:, 512 * b : 512 * (b + 1)], oaccs[(h, b)][: D + 1, :]
        )

    def emit_bank_norm(h, b):
        # Transpose back to [q, 65] IN-PLACE into this bank's retired oacc
        # tile (its own PSUM bank — no slot stolen, no alias with live banks),
        # then normalize by the sums column and store.
        hd = heads[h]
        ot_sb, o_sb, rec = hd["ot"], hd["o"], hd["rec"]
        pso = oaccs[(h, b)][:, : 4 * 65]
        for j in range(4):
            c = 4 * b + j
            nc.tensor.transpose(
                pso[:, 65 * j : 65 * j + 65],
                ot_sb[:, 128 * c : 128 * c + 128],
                id_sb[:65, :65],
            )
        sums = pso.rearrange("p (c d) -> p c d", d=65)[:, :, 64]
        nc.vector.reciprocal(rec[:, 4 * b : 4 * b + 4], sums)
        for j in range(4):
            c = 4 * b + j
            nc.vector.tensor_scalar_mul(
                o_sb[:, 64 * c : 64 * c + 64],
                pso[:, 65 * j : 65 * j + 64],
                rec[:, c : c + 1],
            )
        nc.sync.dma_start(
            o[h, :, 256 * b : 256 * (b + 1)], o_sb[:, 256 * b : 256 * (b + 1)]
        )

    LOOKAHEAD = 3
    load(0, first=True)
    load(1, first=True)
    for i in range(min(LOOKAHEAD, n)):
        emit_S(i)
    deferred = {}  # emit-at chunk idx -> (h, b) norm work
    for i in range(n):
        h, g, kb = chunks[i]
        emit_exp_O(i)
        if i + LOOKAHEAD < n:
            emit_S(i + LOOKAHEAD)
        if i in deferred:
            emit_bank_norm(*deferred.pop(i))
        # Bank completion: pass A finishes banks 0 (kb=3) and 1 (kb=7);
        # pass B finishes banks 2 (kb=11) and 3 (kb=15).
        if kb % 4 == 3 and (g == 0 or kb >= 8):
            b = kb // 4
            emit_bank_copy(h, b)
            deferred[min(i + 2, n - 1)] = (h, b)
            if b == 3 and h + 2 < HPC:
                load(h + 2)
    for i in sorted(deferred):
        emit_bank_norm(*deferred[i])
    deferred.clear()


def _prep_in_maps(Q, K, V):
    Q = np.asarray(Q, dtype=np.float32).reshape(B * NH, T, D)
    K = np.asarray(K, dtype=np.float32).reshape(B * NH, T, D)
    V = np.asarray(V, dtype=np.float32).reshape(B * NH, T, D)

    mask = np.where(
        np.arange(128)[:, None] <= np.arange(128)[None, :], 0.0, NEG
    ).astype(np.float32)
    ident = np.eye(128, dtype=np.float32)

    in_maps = []
    for c in range(NCORES):
        hs = slice(HPC * c, HPC * (c + 1))
        qt = Q[hs].transpose(0, 2, 1)  # [hpc, 64, T]
        kt = K[hs].transpose(0, 2, 1)
        va = np.concatenate(
            [V[hs], np.ones((HPC, T, 1), dtype=np.float32)], axis=-1
        )  # [hpc, T, 65]
        va = va.reshape(HPC, NKB, 128, D + 1).transpose(0, 2, 1, 3)  # [hpc,128,16,65]
        in_maps.append(
            {
                "qt": np.ascontiguousarray(qt),
                "kt": np.ascontiguousarray(kt),
                "v": np.ascontiguousarray(va),
                "mask": mask,
                "ident": ident,
            }
        )
    return in_maps


def _gather(results):
    out = np.empty((B * NH, T, D), dtype=np.float32)
    for c in range(NCORES):
        oc = results[c]["o"]  # [HPC, 128, NKB*D]
        for s in range(HPC):
            out[HPC * c + s] = (
                oc[s].reshape(128, NKB, D).transpose(1, 0, 2).reshape(T, D)
            )
    return out.reshape(B, NH, T, D)


def _run(in_maps, **kwargs):
    nc = _build()
    return run_bass_kernel_spmd(nc, in_maps, core_ids=list(range(NCORES)), **kwargs)


def kernel(Q, K, V):
    in_maps = _prep_in_maps(Q, K, V)
    res = _run(in_maps)
    return _gather(res.results)
